# revision 4
# baseline (speedup 1.0000x reference)
"""3x3 valid conv (single channel) on 8 TRN2 NeuronCores, fp16 I/O.

Strategy (memory-bound => minimize HBM bytes, then keep PE gap-free):
  - All HBM traffic in fp16 (host casts f32->fp16 before sharding, upcasts
    after gather). Halves the 33.8 MB/core f32 traffic to ~16.8 MB/core.
    Max rel err from fp16 in+out is ~8e-4 (gate is 2e-2).
  - Row-wise shard: core i computes output rows [504i, 504i+504) as 4
    strips of 126 rows (each strip = one [128, 8192] input tile; 126 = 128
    - (kh-1)). The global tail of 62 rows (4032..4093) is split by columns
    across all 8 cores (62 x ~1024 each) so no core pays a 5th full-width
    strip of PE streaming. Tail is computed FIRST: its 131 KB input lands
    quickly and its matmuls warm the PE p-state during the strip-0 load.
  - Per strip, conv = 3 banded matmuls per PSUM group: out[m, c] =
    sum_dj (B_dj.T @ X[:, c+dj])[m], with B_dj[k, m] = W[k-m, dj] built on
    the host in fp16. fp16 streams 1 col/cycle; the implicit LDWEIGHTS
    pipelines behind the previous matmul.
  - PSUM as 2 x [128, 2048] f32 mega-tiles (4 banks each), each drained by
    one scalar-engine activation (fused bias, fp16 out).
  - Strip loads ride the SP HWDGE ring as 4 overlapping quarters written
    to disjoint [128, 2050] regions of a [128, 8200] tile, so every
    mega-tile's matmuls depend on exactly one quarter DMA. Stores ride the
    ACT HWDGE ring as [126, 4096] halves. Consts + tail I/O ride the ACT
    ring up front (the single-queue SWDGE ring is too slow to start).
"""

import sys

sys.path.insert(0, "/opt/trn_rl_repo")

import numpy as np
from concourse import bass, mybir
from concourse.bass_utils import run_bass_kernel_spmd
from concourse.tile import TileContext

F16 = mybir.dt.float16
F32 = mybir.dt.float32

H, WIDTH = 4096, 8192
KH, KW = 3, 3
OH, OW = H - KH + 1, WIDTH - KW + 1          # 4094 x 8190
N_CORES = 8
MAIN_RPC = 504                                # main output rows per core
MAIN_IN = MAIN_RPC + KH - 1                   # 506 input rows per core
N_STRIPS = 4                                  # 4 x 126 = 504
TAIL_ROWS = OH - MAIN_RPC * N_CORES           # 62 rows: 4032..4093
TAIL_IN = TAIL_ROWS + KH - 1                  # 64 input rows: 4032..4095
TAIL_CPC = 1024                               # tail cols per core (core 7: 1022)
TAIL_IN_C = TAIL_CPC + KW - 1                 # 1026 input cols

QW = 2050                                     # quarter width incl. kw-1 halo
XS_W = 4 * QW                                 # 8200
MEGA_W = 2048                                 # output cols per PSUM mega-tile
STORE_W = 4096                                # output store chunk width
BIG_N = False                                 # >512-col matmuls fail the walrus
                                              # s3d3_mm_num_elements ISA check
                                              # (one PSUM bank per matmul out)


def _split_multi_waits(nc, max_waits=1):
    # This container's walrus rejects >1 sync-wait command per instruction
    # (CoreV3 setupSyncWait). Tile attaches one wait per producing logical
    # processor to a single instruction; hoist the excess onto same-engine
    # Drain carriers inserted immediately before it.
    for fn in nc.m.functions:
        for bb in fn.blocks:
            out = []
            changed = False
            for inst in bb.instructions:
                si = inst.sync_info
                waits = list(si.on_wait) if si and si.on_wait else []
                if len(waits) > max_waits:
                    rest = waits[max_waits:]
                    for j in range(0, len(rest), max_waits):
                        carrier = mybir.InstDrain(
                            name=nc.get_next_instruction_name(), ins=[], outs=[]
                        )
                        carrier.engine = inst.engine
                        carrier.sync_info = mybir.SyncInfo(
                            on_wait=rest[j : j + max_waits], on_update=[]
                        )
                        out.append(carrier)
                    si.on_wait = waits[:max_waits]
                    changed = True
                out.append(inst)
            if changed:
                bb.instructions = out


def _build(split_waits=True):
    nc = bass.Bass()
    xm = nc.declare_dram_parameter("xm", [MAIN_IN, WIDTH], F16, isOutput=False)
    xt = nc.declare_dram_parameter("xt", [TAIL_IN, TAIL_IN_C], F16, isOutput=False)
    bands = nc.declare_dram_parameter("bands", [128, 3 * 128], F16, isOutput=False)
    bandt = nc.declare_dram_parameter("bandt", [TAIL_IN, 3 * 64], F16, isOutput=False)
    bias = nc.declare_dram_parameter("bias", [128, 1], F32, isOutput=False)
    y = nc.declare_dram_parameter("y", [MAIN_RPC, OW], F16, isOutput=True)
    yt = nc.declare_dram_parameter("yt", [TAIL_ROWS, TAIL_CPC], F16, isOutput=True)

    ident = mybir.ActivationFunctionType.Identity

    with TileContext(nc) as tc:
        with (
            tc.tile_pool(name="const", bufs=1) as cpool,
            tc.tile_pool(name="xin", bufs=2) as xpool,
            tc.tile_pool(name="stage", bufs=3) as spool,
            tc.tile_pool(name="tail", bufs=1) as tpool,
            tc.tile_pool(name="psum", bufs=2, space="PSUM") as ppool,
        ):
            # consts + tail input ride the ACT HWDGE ring, issued first
            band_t = cpool.tile([128, 3 * 128], F16)
            nc.scalar.dma_start(out=band_t[:], in_=bands[:])
            bandt_t = cpool.tile([TAIL_IN, 3 * 64], F16)
            nc.scalar.dma_start(out=bandt_t[:], in_=bandt[:])
            bias_t = cpool.tile([128, 1], F32)
            nc.scalar.dma_start(out=bias_t[:], in_=bias[:])
            xt_t = tpool.tile([TAIL_IN, TAIL_IN_C], F16)
            nc.scalar.dma_start(out=xt_t[:], in_=xt[:])

            # tail first: 62 rows x 1024 cols, K=64 banded matmuls; doubles
            # as PE p-state warmup while strip 0's quarters load
            ps = ppool.tile([128, MEGA_W], F32, tag="ps")
            tail_groups = [(0, 1024)] if BIG_N else [(0, 512), (512, 512)]
            for g0, gn in tail_groups:
                for dj in range(KW):
                    nc.tensor.matmul(
                        ps[:TAIL_ROWS, g0 : g0 + gn],
                        bandt_t[:, dj * 64 : dj * 64 + TAIL_ROWS],
                        xt_t[:, g0 + dj : g0 + dj + gn],
                        start=(dj == 0),
                        stop=(dj == KW - 1),
                    )
            stage_t = tpool.tile([TAIL_ROWS, TAIL_CPC], F16)
            nc.scalar.activation(
                stage_t[:, :],
                ps[:TAIL_ROWS, :TAIL_CPC],
                ident,
                bias=bias_t[:TAIL_ROWS, :],
                scale=1.0,
            )
            nc.scalar.dma_start(out=yt[:, :], in_=stage_t[:, :])

            for strip in range(N_STRIPS):
                r0 = strip * 126
                xs = xpool.tile([128, XS_W], F16, tag="xs")
                # overlapping quarters to disjoint tile regions: mega m's
                # matmuls read only [m*QW, m*QW+2050) = one quarter DMA
                for q in range(4):
                    src0 = q * MEGA_W
                    w = QW if q < 3 else MEGA_W
                    nc.sync.dma_start(
                        out=xs[:, q * QW : q * QW + w],
                        in_=xm[r0 : r0 + 128, src0 : src0 + w],
                    )

                for half in range(2):
                    stage = spool.tile([128, STORE_W], F16, tag="stage")
                    for mt in range(2):
                        m = half * 2 + mt
                        mw = MEGA_W if m < 3 else MEGA_W - 2
                        ps = ppool.tile([128, MEGA_W], F32, tag="ps")
                        groups = (
                            [(0, mw)]
                            if BIG_N
                            else [(j * 512, min(512, mw - j * 512)) for j in range(4)]
                        )
                        for g0, gn in groups:
                            for dj in range(KW):
                                nc.tensor.matmul(
                                    ps[:126, g0 : g0 + gn],
                                    band_t[:, dj * 128 : dj * 128 + 126],
                                    xs[:, m * QW + g0 + dj : m * QW + g0 + dj + gn],
                                    start=(dj == 0),
                                    stop=(dj == KW - 1),
                                )
                        nc.scalar.activation(
                            stage[:126, mt * MEGA_W : mt * MEGA_W + mw],
                            ps[:126, :mw],
                            ident,
                            bias=bias_t[:126, :],
                            scale=1.0,
                        )
                    sw = STORE_W if half == 0 else OW - STORE_W
                    nc.scalar.dma_start(
                        out=y[r0 : r0 + 126, half * STORE_W : half * STORE_W + sw],
                        in_=stage[:126, :sw],
                    )

    if split_waits:
        _split_multi_waits(nc)
    return nc


_NC_CACHE = None


def _get_nc():
    global _NC_CACHE
    if _NC_CACHE is None:
        _NC_CACHE = _build()
    return _NC_CACHE


def _make_host_inputs(X, W, b):
    X16 = np.asarray(X, dtype=np.float16)
    W16 = np.asarray(W, dtype=np.float16)
    b = np.asarray(b, dtype=np.float32)

    bands = np.zeros((128, 3 * 128), dtype=np.float16)
    for dj in range(KW):
        for dk in range(KH):
            mm = np.arange(126)
            bands[mm + dk, dj * 128 + mm] = W16[dk, dj]
    bandt = np.zeros((TAIL_IN, 3 * 64), dtype=np.float16)
    for dj in range(KW):
        for dk in range(KH):
            mm = np.arange(TAIL_ROWS)
            bandt[mm + dk, dj * 64 + mm] = W16[dk, dj]
    bias = np.full((128, 1), float(b[0]), dtype=np.float32)

    in_maps = []
    for i in range(N_CORES):
        r0 = i * MAIN_RPC
        shard = np.ascontiguousarray(X16[r0 : r0 + MAIN_IN])
        c0 = i * TAIL_CPC
        tail = np.zeros((TAIL_IN, TAIL_IN_C), dtype=np.float16)
        cw = min(TAIL_IN_C, WIDTH - c0)
        tail[:, :cw] = X16[OH - TAIL_ROWS : H, c0 : c0 + cw]
        in_maps.append(
            {"xm": shard, "xt": tail, "bands": bands, "bandt": bandt, "bias": bias}
        )
    return in_maps


def _assemble(results):
    out = np.empty((OH, OW), dtype=np.float32)
    for i in range(N_CORES):
        r0 = i * MAIN_RPC
        out[r0 : r0 + MAIN_RPC] = results[i]["y"].astype(np.float32)
        c0 = i * TAIL_CPC
        take = min(TAIL_CPC, OW - c0)
        out[MAIN_RPC * N_CORES :, c0 : c0 + take] = results[i]["yt"][:, :take].astype(
            np.float32
        )
    return out


def run(X, W, b, trace=False):
    nc = _get_nc()
    in_maps = _make_host_inputs(X, W, b)
    res = run_bass_kernel_spmd(nc, in_maps, list(range(N_CORES)), trace=trace)
    return _assemble(res.results), res


def kernel(X, W, b):
    out, _ = run(X, W, b)
    return out


# revision 7
# speedup vs baseline: 1.2167x; 1.2167x over previous
"""3x3 valid conv (single channel) on 8 TRN2 NeuronCores, fp16 I/O.

Strategy (memory-bound => minimize HBM bytes, then keep PE gap-free):
  - All HBM traffic in fp16 (host casts f32->fp16 before sharding, upcasts
    after gather). Halves the 33.8 MB/core f32 traffic to ~16.8 MB/core.
    Max rel err from fp16 in+out is ~8e-4 (gate is 2e-2).
  - Row-wise shard: core i computes output rows [504i, 504i+504) as 4
    strips of 126 rows (each strip = one [128, 8192] input tile; 126 = 128
    - (kh-1)). The global tail of 62 rows (4032..4093) is split by columns
    across all 8 cores (62 x ~1024 each) so no core pays a 5th full-width
    strip of PE streaming. Tail is computed FIRST: its 131 KB input lands
    quickly and its matmuls warm the PE p-state during the strip-0 load.
  - Per strip, conv = 3 banded matmuls per PSUM group: out[m, c] =
    sum_dj (B_dj.T @ X[:, c+dj])[m], with B_dj[k, m] = W[k-m, dj] built on
    the host in fp16. fp16 streams 1 col/cycle; the implicit LDWEIGHTS
    pipelines behind the previous matmul.
  - PSUM as 2 x [128, 2048] f32 mega-tiles (4 banks each), each drained by
    one scalar-engine activation (fused bias, fp16 out).
  - Strip loads ride the SP HWDGE ring as 4 overlapping quarters written
    to disjoint [128, 2050] regions of a [128, 8200] tile, so every
    mega-tile's matmuls depend on exactly one quarter DMA. Stores ride the
    ACT HWDGE ring as [126, 4096] halves. Consts + tail I/O ride the ACT
    ring up front (the single-queue SWDGE ring is too slow to start).
"""

import sys

sys.path.insert(0, "/opt/trn_rl_repo")

import numpy as np
from concourse import bass, mybir
from concourse.bass_utils import run_bass_kernel_spmd
from concourse.tile import TileContext

F16 = mybir.dt.float16
F32 = mybir.dt.float32

H, WIDTH = 4096, 8192
KH, KW = 3, 3
OH, OW = H - KH + 1, WIDTH - KW + 1          # 4094 x 8190
N_CORES = 8
MAIN_RPC = 504                                # main output rows per core
MAIN_IN = MAIN_RPC + KH - 1                   # 506 input rows per core
N_STRIPS = 4                                  # 4 x 126 = 504
TAIL_ROWS = OH - MAIN_RPC * N_CORES           # 62 rows: 4032..4093
TAIL_IN = TAIL_ROWS + KH - 1                  # 64 input rows: 4032..4095
TAIL_CPC = 1024                               # tail cols per core (core 7: 1022)
TAIL_IN_C = TAIL_CPC + KW - 1                 # 1026 input cols

QW = 2050                                     # quarter width incl. kw-1 halo
XS_W = 4 * QW                                 # 8200
MEGA_W = 2048                                 # output cols per PSUM mega-tile
STORE_W = 4096                                # output store chunk width
BIG_N = False                                 # >512-col matmuls fail the walrus
                                              # s3d3_mm_num_elements ISA check
                                              # (one PSUM bank per matmul out)
WARMUP = 16                                   # PE p-state warmup matmuls


def _split_multi_waits(nc, max_waits=1):
    # This container's walrus rejects >1 sync-wait command per instruction
    # (CoreV3 setupSyncWait). Tile attaches one wait per producing logical
    # processor to a single instruction; hoist the excess onto same-engine
    # Drain carriers inserted immediately before it.
    for fn in nc.m.functions:
        for bb in fn.blocks:
            out = []
            changed = False
            for inst in bb.instructions:
                si = inst.sync_info
                waits = list(si.on_wait) if si and si.on_wait else []
                if len(waits) > max_waits:
                    rest = waits[max_waits:]
                    for j in range(0, len(rest), max_waits):
                        carrier = mybir.InstDrain(
                            name=nc.get_next_instruction_name(), ins=[], outs=[]
                        )
                        carrier.engine = inst.engine
                        carrier.sync_info = mybir.SyncInfo(
                            on_wait=rest[j : j + max_waits], on_update=[]
                        )
                        out.append(carrier)
                    si.on_wait = waits[:max_waits]
                    changed = True
                out.append(inst)
            if changed:
                bb.instructions = out


def _build(split_waits=True):
    nc = bass.Bass()
    xm = nc.declare_dram_parameter("xm", [MAIN_IN, WIDTH], F16, isOutput=False)
    xt = nc.declare_dram_parameter("xt", [TAIL_IN, TAIL_IN_C], F16, isOutput=False)
    bands = nc.declare_dram_parameter("bands", [128, 3 * 128], F16, isOutput=False)
    bandt = nc.declare_dram_parameter("bandt", [TAIL_IN, 3 * 64], F16, isOutput=False)
    bias = nc.declare_dram_parameter("bias", [128, 1], F32, isOutput=False)
    y = nc.declare_dram_parameter("y", [MAIN_RPC, OW], F16, isOutput=True)
    yt = nc.declare_dram_parameter("yt", [TAIL_ROWS, TAIL_CPC], F16, isOutput=True)

    ident = mybir.ActivationFunctionType.Identity

    with TileContext(nc) as tc:
        with (
            tc.tile_pool(name="const", bufs=1) as cpool,
            tc.tile_pool(name="xin", bufs=2) as xpool,
            tc.tile_pool(name="stage", bufs=3) as spool,
            tc.tile_pool(name="tail", bufs=1) as tpool,
            tc.tile_pool(name="psum", bufs=2, space="PSUM") as ppool,
        ):
            # consts + tail input ride the ACT HWDGE ring, issued first
            band_t = cpool.tile([128, 3 * 128], F16)
            nc.scalar.dma_start(out=band_t[:], in_=bands[:])
            bias_t = cpool.tile([128, 1], F32)
            nc.scalar.dma_start(out=bias_t[:], in_=bias[:])
            bandt_t = cpool.tile([TAIL_IN, 3 * 64], F16)
            nc.scalar.dma_start(out=bandt_t[:], in_=bandt[:])
            xt_t = tpool.tile([TAIL_IN, TAIL_IN_C], F16)
            nc.scalar.dma_start(out=xt_t[:], in_=xt[:])

            # PE p-state warmup on a memset tile: no DMA dependency, so the
            # PE ramps to full clock during the ~10us DMA-ring startup. The
            # warmup PSUM tile is never drained (pool rotation reclaims it).
            wsrc = cpool.tile([128, 512], F16)
            nc.vector.memset(wsrc[:], 0.0)
            wps = ppool.tile([128, MEGA_W], F32, tag="ps")
            for wi in range(WARMUP):
                nc.tensor.matmul(
                    wps[:126, (wi % 4) * 512 : (wi % 4) * 512 + 512],
                    wsrc[:, :126],
                    wsrc[:, :512],
                    start=True,
                    stop=True,
                )

            for strip in range(N_STRIPS):
                r0 = strip * 126
                xs = xpool.tile([128, XS_W], F16, tag="xs")
                # overlapping quarters to disjoint tile regions: mega m's
                # matmuls read only [m*QW, m*QW+2050) = one quarter DMA
                for q in range(4):
                    src0 = q * MEGA_W
                    w = QW if q < 3 else MEGA_W
                    nc.sync.dma_start(
                        out=xs[:, q * QW : q * QW + w],
                        in_=xm[r0 : r0 + 128, src0 : src0 + w],
                    )

                for half in range(2):
                    stage = spool.tile([128, STORE_W], F16, tag="stage")
                    for mt in range(2):
                        m = half * 2 + mt
                        mw = MEGA_W if m < 3 else MEGA_W - 2
                        ps = ppool.tile([128, MEGA_W], F32, tag="ps")
                        groups = (
                            [(0, mw)]
                            if BIG_N
                            else [(j * 512, min(512, mw - j * 512)) for j in range(4)]
                        )
                        for g0, gn in groups:
                            for dj in range(KW):
                                nc.tensor.matmul(
                                    ps[:126, g0 : g0 + gn],
                                    band_t[:, dj * 128 : dj * 128 + 126],
                                    xs[:, m * QW + g0 + dj : m * QW + g0 + dj + gn],
                                    start=(dj == 0),
                                    stop=(dj == KW - 1),
                                )
                        nc.scalar.activation(
                            stage[:126, mt * MEGA_W : mt * MEGA_W + mw],
                            ps[:126, :mw],
                            ident,
                            bias=bias_t[:126, :],
                            scale=1.0,
                        )
                    sw = STORE_W if half == 0 else OW - STORE_W
                    nc.scalar.dma_start(
                        out=y[r0 : r0 + 126, half * STORE_W : half * STORE_W + sw],
                        in_=stage[:126, :sw],
                    )

            # tail last: 62 rows x 1024 cols, K=64 banded matmuls (its input
            # landed long ago on the ACT ring)
            ps = ppool.tile([128, MEGA_W], F32, tag="ps")
            for g0, gn in [(0, 512), (512, 512)]:
                for dj in range(KW):
                    nc.tensor.matmul(
                        ps[:TAIL_ROWS, g0 : g0 + gn],
                        bandt_t[:, dj * 64 : dj * 64 + TAIL_ROWS],
                        xt_t[:, g0 + dj : g0 + dj + gn],
                        start=(dj == 0),
                        stop=(dj == KW - 1),
                    )
            stage_t = tpool.tile([TAIL_ROWS, TAIL_CPC], F16)
            nc.scalar.activation(
                stage_t[:, :],
                ps[:TAIL_ROWS, :TAIL_CPC],
                ident,
                bias=bias_t[:TAIL_ROWS, :],
                scale=1.0,
            )
            nc.scalar.dma_start(out=yt[:, :], in_=stage_t[:, :])

    if split_waits:
        _split_multi_waits(nc)
    return nc


_NC_CACHE = None


def _get_nc():
    global _NC_CACHE
    if _NC_CACHE is None:
        _NC_CACHE = _build()
    return _NC_CACHE


def _make_host_inputs(X, W, b):
    X16 = np.asarray(X, dtype=np.float16)
    W16 = np.asarray(W, dtype=np.float16)
    b = np.asarray(b, dtype=np.float32)

    bands = np.zeros((128, 3 * 128), dtype=np.float16)
    for dj in range(KW):
        for dk in range(KH):
            mm = np.arange(126)
            bands[mm + dk, dj * 128 + mm] = W16[dk, dj]
    bandt = np.zeros((TAIL_IN, 3 * 64), dtype=np.float16)
    for dj in range(KW):
        for dk in range(KH):
            mm = np.arange(TAIL_ROWS)
            bandt[mm + dk, dj * 64 + mm] = W16[dk, dj]
    bias = np.full((128, 1), float(b[0]), dtype=np.float32)

    in_maps = []
    for i in range(N_CORES):
        r0 = i * MAIN_RPC
        shard = np.ascontiguousarray(X16[r0 : r0 + MAIN_IN])
        c0 = i * TAIL_CPC
        tail = np.zeros((TAIL_IN, TAIL_IN_C), dtype=np.float16)
        cw = min(TAIL_IN_C, WIDTH - c0)
        tail[:, :cw] = X16[OH - TAIL_ROWS : H, c0 : c0 + cw]
        in_maps.append(
            {"xm": shard, "xt": tail, "bands": bands, "bandt": bandt, "bias": bias}
        )
    return in_maps


def _assemble(results):
    out = np.empty((OH, OW), dtype=np.float32)
    for i in range(N_CORES):
        r0 = i * MAIN_RPC
        out[r0 : r0 + MAIN_RPC] = results[i]["y"].astype(np.float32)
        c0 = i * TAIL_CPC
        take = min(TAIL_CPC, OW - c0)
        out[MAIN_RPC * N_CORES :, c0 : c0 + take] = results[i]["yt"][:, :take].astype(
            np.float32
        )
    return out


def run(X, W, b, trace=False):
    nc = _get_nc()
    in_maps = _make_host_inputs(X, W, b)
    res = run_bass_kernel_spmd(nc, in_maps, list(range(N_CORES)), trace=trace)
    return _assemble(res.results), res


def kernel(X, W, b):
    out, _ = run(X, W, b)
    return out
